# revision 3
# baseline (speedup 1.0000x reference)
"""GQA attention block (RoPE + causal attention + output proj) on 8 TRN2 NeuronCores.

Sharding: batch (B=2) x kv-head-group (KVH=4) -> 8 cores.
Core c handles batch b=c//4, kv group g=c%4 (q heads 4g..4g+3, kv head g).
Per-core tensor-parallel attention; AllGather of per-head outputs within each
batch's 4-core group; column-split wo after the gather.

All matmuls run in bf16 (fp32 PSUM accumulation). Layouts are transposed
([feature, token]) so Q/K/V projections, scores (computed as S^T = K-stationary),
and PV all feed the TensorEngine without transposes; softmax runs without
max-subtraction (logits are provably small for this problem's scale).
"""

import sys

if "/opt/trn_rl_repo" not in sys.path:
    sys.path.insert(0, "/opt/trn_rl_repo")

import numpy as np
import ml_dtypes

import concourse.bass as bass
import concourse.mybir as mybir
import concourse.tile as tile
from concourse import bacc
from concourse.bass_utils import run_bass_kernel_spmd

BF16 = ml_dtypes.bfloat16

B, S, HID = 2, 2048, 1024
H, KVH, D = 16, 4, 64
G = H // KVH
N_CORES = 8
SPAN = 512
NSPAN = S // SPAN  # 4
NCH = HID // 128  # 8 contraction chunks
NKT = S // 128  # 16 k-tiles
F32 = mybir.dt.float32
BF = mybir.dt.bfloat16

TRACE = False
_CACHED = {}


def _build_nc():
    nc = bacc.Bacc("TRN2", target_bir_lowering=False, debug=False, num_devices=N_CORES)

    xT = nc.dram_tensor("xT", [HID, S], BF, kind="ExternalInput")
    wq = nc.dram_tensor("wq", [HID, 256], BF, kind="ExternalInput")
    wkv = nc.dram_tensor("wkv", [HID, 128], BF, kind="ExternalInput")
    wo = nc.dram_tensor("wo", [HID, 256], BF, kind="ExternalInput")
    c2 = nc.dram_tensor("c2", [128, S], F32, kind="ExternalInput")
    s2 = nc.dram_tensor("s2", [128, S], F32, kind="ExternalInput")
    c1 = nc.dram_tensor("c1", [64, S], F32, kind="ExternalInput")
    s1 = nc.dram_tensor("s1", [64, S], F32, kind="ExternalInput")
    ident = nc.dram_tensor("ident", [128, 128], BF, kind="ExternalInput")
    out = nc.dram_tensor("out", [256, S], F32, kind="ExternalOutput")

    EXP = mybir.ActivationFunctionType.Exp
    LN = mybir.ActivationFunctionType.Ln

    with tile.TileContext(nc) as tc:
        with (
            tc.tile_pool(name="main", bufs=1) as main,
            tc.tile_pool(name="dramp", bufs=1, space="DRAM") as dramp,
        ):
            # ---- persistent SBUF ----
            xT_sb = main.tile([128, NCH, S], BF, name="xT_sb")
            wq_sb = main.tile([128, NCH, 256], BF, name="wq_sb")
            wkv_sb = main.tile([128, NCH, 128], BF, name="wkv_sb")
            wo_sb = main.tile([128, NCH, 256], BF, name="wo_sb")
            c2_sb = main.tile([128, S], F32, name="c2_sb")
            s2_sb = main.tile([128, S], F32, name="s2_sb")
            c1_sb = main.tile([64, S], F32, name="c1_sb")
            s1_sb = main.tile([64, S], F32, name="s1_sb")
            ident_sb = main.tile([128, 128], BF, name="ident_sb")
            qT0_sb = main.tile([128, S], BF, name="qT0_sb")
            qT1_sb = main.tile([128, S], BF, name="qT1_sb")
            kT2_sb = main.tile([128, S], BF, name="kT2_sb")
            vT_sb = main.tile([64, S], BF, name="vT_sb")
            vaug_sb = main.tile([128, NKT, 65], BF, name="vaug_sb")
            ones_sb = main.tile([1, 64], BF, name="ones_sb")
            qT_sb = [qT0_sb, qT1_sb]

            for k in range(NCH):
                nc.sync.dma_start(xT_sb[:, k, :], xT[128 * k : 128 * k + 128, :])
                nc.sync.dma_start(wq_sb[:, k, :], wq[128 * k : 128 * k + 128, :])
                nc.sync.dma_start(wkv_sb[:, k, :], wkv[128 * k : 128 * k + 128, :])
                nc.sync.dma_start(wo_sb[:, k, :], wo[128 * k : 128 * k + 128, :])
            nc.sync.dma_start(c2_sb[:], c2[:])
            nc.sync.dma_start(s2_sb[:], s2[:])
            nc.sync.dma_start(c1_sb[:], c1[:])
            nc.sync.dma_start(s1_sb[:], s1[:])
            nc.sync.dma_start(ident_sb[:], ident[:])
            nc.vector.memset(ones_sb[:], 1.0)

            # ---- phase 1: Q/K/V projections (transposed layout) + RoPE ----
            with (
                tc.tile_pool(name="psA", bufs=2, space="PSUM") as psA,
                tc.tile_pool(name="ropep", bufs=2) as ropep,
            ):
                for p in range(2):
                    qp = psA.tile([128, S], F32, tag="qkv", name=f"qp{p}")
                    for sp in range(NSPAN):
                        for k in range(NCH):
                            nc.tensor.matmul(
                                qp[:, SPAN * sp : SPAN * (sp + 1)],
                                wq_sb[:, k, 128 * p : 128 * (p + 1)],
                                xT_sb[:, k, SPAN * sp : SPAN * (sp + 1)],
                                start=(k == 0),
                                stop=(k == NCH - 1),
                            )
                    tcos = ropep.tile([128, S], F32, tag="tcos", name=f"tcos{p}")
                    tsin = ropep.tile([128, S], F32, tag="tsin", name=f"tsin{p}")
                    nc.vector.tensor_mul(tcos[:], qp[:], c2_sb[:])
                    for dst, src in ((0, 32), (32, 0), (64, 96), (96, 64)):
                        nc.vector.tensor_mul(
                            tsin[dst : dst + 32, :],
                            qp[src : src + 32, :],
                            s2_sb[dst : dst + 32, :],
                        )
                    nc.vector.tensor_add(qT_sb[p][:], tcos[:], tsin[:])

                kvp = psA.tile([128, S], F32, tag="qkv", name="kvp")
                for sp in range(NSPAN):
                    for k in range(NCH):
                        nc.tensor.matmul(
                            kvp[:, SPAN * sp : SPAN * (sp + 1)],
                            wkv_sb[:, k, :],
                            xT_sb[:, k, SPAN * sp : SPAN * (sp + 1)],
                            start=(k == 0),
                            stop=(k == NCH - 1),
                        )
                tcosk = ropep.tile([64, S], F32, tag="tcos", name="tcosk")
                tsink = ropep.tile([64, S], F32, tag="tsin", name="tsink")
                nc.vector.tensor_mul(tcosk[:], kvp[0:64, :], c1_sb[:])
                for dst, src in ((0, 32), (32, 0)):
                    nc.vector.tensor_mul(
                        tsink[dst : dst + 32, :],
                        kvp[src : src + 32, :],
                        s1_sb[dst : dst + 32, :],
                    )
                nc.vector.tensor_add(kT2_sb[0:64, :], tcosk[:], tsink[:])
                nc.vector.tensor_copy(kT2_sb[64:128, :], kT2_sb[0:64, :])
                nc.vector.tensor_copy(vT_sb[:], kvp[64:128, :])

            # ---- phase 2: transpose V to [token, d] and append ones column ----
            with tc.tile_pool(name="psT", bufs=2, space="PSUM") as psT:
                for t in range(NKT):
                    trp = psT.tile([128, 64], BF, tag="tr", name=f"tr{t}")
                    nc.tensor.transpose(
                        trp[:], vT_sb[:, 128 * t : 128 * (t + 1)], ident_sb[0:64, 0:64]
                    )
                    nc.vector.tensor_copy(vaug_sb[:, t, 0:64], trp[:])
                nc.vector.memset(vaug_sb[:, :, 64:65], 1.0)

            # ---- phase 3: attention spans + AllGather + output projection ----
            with (
                tc.tile_pool(name="psS", bufs=2, space="PSUM") as psS,
                tc.tile_pool(name="psO", bufs=1, space="PSUM") as psO,
                tc.tile_pool(name="pp", bufs=3) as pp,
                tc.tile_pool(name="work", bufs=2) as work,
            ):
                group = [c for c in range(4)]
                rg = [group, [c + 4 for c in group]]
                for J in range(NSPAN):
                    q0 = SPAN * J
                    nkt_j = 4 * (J + 1)
                    opsum = psO.tile([128, 4 * SPAN], F32, tag="o", name=f"opsum{J}")

                    for j in range(nkt_j):
                        # pair 0 = heads 0,1 ; pair 1 = heads 2,3
                        for pr in range(2):
                            sps = psS.tile(
                                [128, 2 * SPAN], F32, tag="s", name=f"s{J}_{j}_{pr}"
                            )
                            pt = pp.tile(
                                [128, 2 * SPAN], BF, tag="p", name=f"p{J}_{j}_{pr}"
                            )
                            src = qT_sb[pr]
                            nc.tensor.matmul(
                                sps[:, 0:SPAN],
                                kT2_sb[0:64, 128 * j : 128 * (j + 1)],
                                src[0:64, q0 : q0 + SPAN],
                                start=True,
                                stop=True,
                            )
                            nc.tensor.matmul(
                                sps[:, SPAN : 2 * SPAN],
                                kT2_sb[64:128, 128 * j : 128 * (j + 1)],
                                src[64:128, q0 : q0 + SPAN],
                                start=True,
                                stop=True,
                            )
                            nc.scalar.activation(pt[:], sps[:], EXP)
                            if j >= 4 * J:
                                jj = j - 4 * J
                                for hh in range(2):
                                    nc.gpsimd.affine_select(
                                        pt[:, SPAN * hh : SPAN * (hh + 1)],
                                        pt[:, SPAN * hh : SPAN * (hh + 1)],
                                        pattern=[[1, SPAN]],
                                        compare_op=mybir.AluOpType.is_ge,
                                        fill=0.0,
                                        base=-128 * jj,
                                        channel_multiplier=-1,
                                    )
                            for hh in range(2):
                                h = 2 * pr + hh
                                nc.tensor.matmul(
                                    opsum[0:65, SPAN * h : SPAN * (h + 1)],
                                    vaug_sb[:, j, :],
                                    pt[:, SPAN * hh : SPAN * (hh + 1)],
                                    start=(j == 0),
                                    stop=(j == nkt_j - 1),
                                )

                    # normalization: recip of denominators via exp(-ln(d))
                    lnb = work.tile([1, 4 * SPAN], F32, tag="lnb", name=f"lnb{J}")
                    nc.scalar.activation(lnb[:], opsum[64:65, :], LN)
                    recip = work.tile([1, 4 * SPAN], BF, tag="recip", name=f"recip{J}")
                    nc.scalar.activation(recip[:], lnb[:], EXP, scale=-1.0)

                    agin = dramp.tile([256, SPAN], BF, name=f"agin{J}")
                    agout = dramp.tile([4 * 256, SPAN], BF, name=f"agout{J}")
                    for h in range(4):
                        bc = psS.tile([64, SPAN], F32, tag="s", name=f"bc{J}_{h}")
                        nc.tensor.matmul(
                            bc[:],
                            ones_sb[:],
                            recip[0:1, SPAN * h : SPAN * (h + 1)],
                            start=True,
                            stop=True,
                        )
                        bcs = work.tile([64, SPAN], BF, tag="bcs", name=f"bcs{J}_{h}")
                        nc.vector.tensor_copy(bcs[:], bc[:])
                        onrm = work.tile([64, SPAN], BF, tag="onrm", name=f"on{J}_{h}")
                        nc.vector.tensor_mul(
                            onrm[:], opsum[0:64, SPAN * h : SPAN * (h + 1)], bcs[:]
                        )
                        nc.sync.dma_start(agin[64 * h : 64 * (h + 1), :], onrm[:])

                    nc.gpsimd.collective_compute(
                        "AllGather",
                        mybir.AluOpType.bypass,
                        replica_groups=rg,
                        ins=[agin[:].opt()],
                        outs=[agout[:].opt()],
                    )

                    ofull = work.tile([128, NCH, SPAN], BF, tag="ofull", name=f"of{J}")
                    for k in range(NCH):
                        nc.sync.dma_start(
                            ofull[:, k, :], agout[128 * k : 128 * (k + 1), :]
                        )
                    for half in range(2):
                        po = psS.tile([128, SPAN], F32, tag="s", name=f"po{J}_{half}")
                        for k in range(NCH):
                            nc.tensor.matmul(
                                po[:],
                                wo_sb[:, k, 128 * half : 128 * (half + 1)],
                                ofull[:, k, :],
                                start=(k == 0),
                                stop=(k == NCH - 1),
                            )
                        outT = work.tile([128, SPAN], F32, tag="outT", name=f"ot{J}_{half}")
                        nc.vector.tensor_copy(outT[:], po[:])
                        nc.sync.dma_start(
                            out[128 * half : 128 * (half + 1), q0 : q0 + SPAN], outT[:]
                        )

    nc.finalize()
    return nc


def _host_inputs(x, cos, sin, wq, wk, wv, wo):
    cosT = np.ascontiguousarray(cos.T).astype(np.float32)  # [64, S]
    sinT = np.ascontiguousarray(sin.T).astype(np.float32)
    s1n = np.concatenate([-sinT[0:32], sinT[32:64]], axis=0)  # [64, S]
    c2n = np.concatenate([cosT, cosT], axis=0)  # [128, S]
    s2n = np.concatenate([s1n, s1n], axis=0)
    ident = np.eye(128, dtype=BF16)

    in_maps = []
    for c in range(N_CORES):
        b, g = c // 4, c % 4
        xT = np.ascontiguousarray(x[b].T).astype(BF16)
        wq_c = np.ascontiguousarray(wq[:, 256 * g : 256 * (g + 1)] / 8.0).astype(BF16)
        wkv_c = np.ascontiguousarray(
            np.concatenate(
                [wk[:, 64 * g : 64 * (g + 1)], wv[:, 64 * g : 64 * (g + 1)]], axis=1
            )
        ).astype(BF16)
        wo_c = np.ascontiguousarray(wo[:, 256 * g : 256 * (g + 1)]).astype(BF16)
        in_maps.append(
            {
                "xT": xT,
                "wq": wq_c,
                "wkv": wkv_c,
                "wo": wo_c,
                "c2": c2n,
                "s2": s2n,
                "c1": cosT,
                "s1": s1n,
                "ident": ident,
            }
        )
    return in_maps


def kernel(x, cos, sin, wq, wk, wv, wo):
    if "nc" not in _CACHED:
        _CACHED["nc"] = _build_nc()
    nc = _CACHED["nc"]
    in_maps = _host_inputs(
        np.asarray(x, np.float32),
        np.asarray(cos, np.float32),
        np.asarray(sin, np.float32),
        np.asarray(wq, np.float32),
        np.asarray(wk, np.float32),
        np.asarray(wv, np.float32),
        np.asarray(wo, np.float32),
    )
    res = run_bass_kernel_spmd(
        nc, in_maps, core_ids=list(range(N_CORES)), trace=TRACE
    )
    _CACHED["last_result"] = res
    out = np.empty((B, S, HID), dtype=np.float32)
    for c in range(N_CORES):
        b, g = c // 4, c % 4
        out[b, :, 256 * g : 256 * (g + 1)] = res.results[c]["out"].T
    return out


# revision 5
# speedup vs baseline: 1.3268x; 1.3268x over previous
"""GQA attention block (RoPE + causal attention + output proj) on 8 TRN2 NeuronCores.

Sharding: batch (B=2) x kv-head-group (KVH=4) -> 8 cores.
Core c handles batch b=c//4, kv group g=c%4 (q heads 4g..4g+3, kv head g).
Per-core tensor-parallel attention; AllGather of per-head outputs within each
batch's 4-core group; column-split wo after the gather.

All matmuls run in bf16 (fp32 PSUM accumulation). Layouts are transposed
([feature, token]) so Q/K/V projections, scores (computed as S^T = K-stationary),
and PV all feed the TensorEngine without transposes; softmax runs without
max-subtraction (logits are provably small for this problem's scale).

Pipelining: PV runs one k-tile behind scores/exp; each span's output projection
is deferred until after the next span's attention so the AllGather latency hides
behind compute.
"""

import sys

if "/opt/trn_rl_repo" not in sys.path:
    sys.path.insert(0, "/opt/trn_rl_repo")

import numpy as np
import ml_dtypes

import concourse.bass as bass
import concourse.mybir as mybir
import concourse.tile as tile
from concourse import bacc
from concourse.bass_utils import run_bass_kernel_spmd

BF16 = ml_dtypes.bfloat16

B, S, HID = 2, 2048, 1024
H, KVH, D = 16, 4, 64
G = H // KVH
N_CORES = 8
SPAN = 512
NSPAN = S // SPAN  # 4
NCH = HID // 128  # 8 contraction chunks
NKT = S // 128  # 16 k-tiles
F32 = mybir.dt.float32
BF = mybir.dt.bfloat16

TRACE = False
_CACHED = {}


def _build_nc():
    nc = bacc.Bacc("TRN2", target_bir_lowering=False, debug=False, num_devices=N_CORES)

    xT = nc.dram_tensor("xT", [HID, S], BF, kind="ExternalInput")
    wq = nc.dram_tensor("wq", [HID, 256], BF, kind="ExternalInput")
    wkv = nc.dram_tensor("wkv", [HID, 128], BF, kind="ExternalInput")
    wo = nc.dram_tensor("wo", [HID, 256], BF, kind="ExternalInput")
    c2 = nc.dram_tensor("c2", [128, S], F32, kind="ExternalInput")
    s2 = nc.dram_tensor("s2", [128, S], F32, kind="ExternalInput")
    c1 = nc.dram_tensor("c1", [64, S], F32, kind="ExternalInput")
    s1 = nc.dram_tensor("s1", [64, S], F32, kind="ExternalInput")
    ident = nc.dram_tensor("ident", [128, 128], BF, kind="ExternalInput")
    out = nc.dram_tensor("out", [256, S], F32, kind="ExternalOutput")

    EXP = mybir.ActivationFunctionType.Exp
    LN = mybir.ActivationFunctionType.Ln

    with tile.TileContext(nc) as tc:
        with (
            tc.tile_pool(name="main", bufs=1) as main,
            tc.tile_pool(name="dramp", bufs=1, space="DRAM") as dramp,
        ):
            # ---- persistent SBUF; per-chunk input tiles so compute can start
            # as soon as each chunk's DMA lands ----
            xT_sb = [main.tile([128, S], BF, name=f"xT{k}") for k in range(NCH)]
            wq_sb = [main.tile([128, 256], BF, name=f"wq{k}") for k in range(NCH)]
            wkv_sb = [main.tile([128, 128], BF, name=f"wkv{k}") for k in range(NCH)]
            wo_sb = [main.tile([128, 256], BF, name=f"wo{k}") for k in range(NCH)]
            c2_sb = main.tile([128, S], F32, name="c2_sb")
            s2_sb = main.tile([128, S], F32, name="s2_sb")
            c1_sb = main.tile([64, S], F32, name="c1_sb")
            s1_sb = main.tile([64, S], F32, name="s1_sb")
            ident_sb = main.tile([128, 128], BF, name="ident_sb")
            qT0_sb = main.tile([128, S], BF, name="qT0_sb")
            qT1_sb = main.tile([128, S], BF, name="qT1_sb")
            kT2_sb = main.tile([128, S], BF, name="kT2_sb")
            vT_sb = main.tile([64, S], BF, name="vT_sb")
            vaug_sb = main.tile([128, NKT, 65], BF, name="vaug_sb")
            ones_sb = main.tile([1, 64], BF, name="ones_sb")
            qT_sb = [qT0_sb, qT1_sb]

            for k in range(NCH):
                nc.sync.dma_start(wkv_sb[k][:], wkv[128 * k : 128 * k + 128, :])
            for k in range(NCH):
                nc.sync.dma_start(xT_sb[k][:], xT[128 * k : 128 * k + 128, :])
            for k in range(NCH):
                nc.sync.dma_start(wq_sb[k][:], wq[128 * k : 128 * k + 128, :])
            nc.sync.dma_start(c1_sb[:], c1[:])
            nc.sync.dma_start(s1_sb[:], s1[:])
            nc.sync.dma_start(c2_sb[:], c2[:])
            nc.sync.dma_start(s2_sb[:], s2[:])
            nc.sync.dma_start(ident_sb[:], ident[:])
            for k in range(NCH):
                nc.sync.dma_start(wo_sb[k][:], wo[128 * k : 128 * k + 128, :])
            nc.vector.memset(ones_sb[:], 1.0)

            # ---- phase 1: projections (transposed layout) + RoPE; KV first so
            # the V-transpose can run while the Q projections are still going ----
            HS = S // 2  # phase-1 half-sequence granularity (2 PSUM banks)
            with (
                tc.tile_pool(name="psA", bufs=2, space="PSUM") as psA,
                tc.tile_pool(name="ropep", bufs=2) as ropep,
                tc.tile_pool(name="psT", bufs=2, space="PSUM") as psT,
            ):
                for hf in range(2):
                    f0 = HS * hf
                    kvp = psA.tile([128, HS], F32, tag="qkv", name=f"kvp{hf}")
                    for sp in range(2):
                        for k in range(NCH):
                            nc.tensor.matmul(
                                kvp[:, SPAN * sp : SPAN * (sp + 1)],
                                wkv_sb[k][:],
                                xT_sb[k][:, f0 + SPAN * sp : f0 + SPAN * (sp + 1)],
                                start=(k == 0),
                                stop=(k == NCH - 1),
                            )
                    tcosk = ropep.tile([64, HS], F32, tag="tcos", name=f"tcosk{hf}")
                    tsink = ropep.tile([64, HS], F32, tag="tsin", name=f"tsink{hf}")
                    nc.vector.tensor_mul(tcosk[:], kvp[0:64, :], c1_sb[:, f0 : f0 + HS])
                    for dst, src in ((0, 32), (32, 0)):
                        nc.vector.tensor_mul(
                            tsink[dst : dst + 32, :],
                            kvp[src : src + 32, :],
                            s1_sb[dst : dst + 32, f0 : f0 + HS],
                        )
                    nc.vector.tensor_add(
                        kT2_sb[0:64, f0 : f0 + HS], tcosk[:], tsink[:]
                    )
                    nc.vector.tensor_copy(
                        kT2_sb[64:128, f0 : f0 + HS], kT2_sb[0:64, f0 : f0 + HS]
                    )
                    nc.vector.tensor_copy(vT_sb[:, f0 : f0 + HS], kvp[64:128, :])
                    # V transpose to [token, d] for this half
                    for t in range(8 * hf, 8 * hf + 8):
                        trp = psT.tile([128, 64], BF, tag="tr", name=f"tr{t}")
                        nc.tensor.transpose(
                            trp[:],
                            vT_sb[:, 128 * t : 128 * (t + 1)],
                            ident_sb[0:64, 0:64],
                        )
                        nc.vector.tensor_copy(vaug_sb[:, t, 0:64], trp[:])
                nc.vector.memset(vaug_sb[:, :, 64:65], 1.0)

                for p in range(2):
                    for hf in range(2):
                        f0 = HS * hf
                        qp = psA.tile([128, HS], F32, tag="qkv", name=f"qp{p}_{hf}")
                        for sp in range(2):
                            for k in range(NCH):
                                nc.tensor.matmul(
                                    qp[:, SPAN * sp : SPAN * (sp + 1)],
                                    wq_sb[k][:, 128 * p : 128 * (p + 1)],
                                    xT_sb[k][:, f0 + SPAN * sp : f0 + SPAN * (sp + 1)],
                                    start=(k == 0),
                                    stop=(k == NCH - 1),
                                )
                        tcos = ropep.tile([128, HS], F32, tag="tcos", name=f"tc{p}{hf}")
                        tsin = ropep.tile([128, HS], F32, tag="tsin", name=f"ts{p}{hf}")
                        nc.vector.tensor_mul(tcos[:], qp[:], c2_sb[:, f0 : f0 + HS])
                        for dst, src in ((0, 32), (32, 0), (64, 96), (96, 64)):
                            nc.vector.tensor_mul(
                                tsin[dst : dst + 32, :],
                                qp[src : src + 32, :],
                                s2_sb[dst : dst + 32, f0 : f0 + HS],
                            )
                        nc.vector.tensor_add(
                            qT_sb[p][:, f0 : f0 + HS], tcos[:], tsin[:]
                        )

            # ---- phase 3: attention spans, AllGather, output projection ----
            with (
                tc.tile_pool(name="psS", bufs=2, space="PSUM") as psS,
                tc.tile_pool(name="psO", bufs=1, space="PSUM") as psO,
                tc.tile_pool(name="pp", bufs=5) as pp,
                tc.tile_pool(name="work", bufs=2) as work,
            ):
                rg = [[0, 1, 2, 3], [4, 5, 6, 7]]
                pending_oproj = None

                for J in range(NSPAN):
                    q0 = SPAN * J
                    nkt_j = 4 * (J + 1)
                    opsum = psO.tile([128, 4 * SPAN], F32, tag="o", name=f"opsum{J}")

                    prev_pv = None  # (j, [pt_pr0, pt_pr1], col offset)

                    def emit_pv(j, pts, off):
                        for pr in range(2):
                            for hh in range(2):
                                h = 2 * pr + hh
                                nc.tensor.matmul(
                                    opsum[0:65, SPAN * h + off : SPAN * (h + 1)],
                                    vaug_sb[:, j, :],
                                    pts[pr][:, SPAN * hh + off : SPAN * (hh + 1)],
                                    start=(j == 0),
                                    stop=(j == nkt_j - 1),
                                )

                    for j in range(nkt_j):
                        jj = j - 4 * J  # >= 0 on causal-boundary k-tiles
                        off = 128 * jj if jj > 0 else 0
                        pts = []
                        for pr in range(2):
                            sps = psS.tile(
                                [128, 2 * SPAN], F32, tag="s", name=f"s{J}_{j}_{pr}"
                            )
                            pt = pp.tile(
                                [128, 2 * SPAN], BF, tag="p", name=f"p{J}_{j}_{pr}"
                            )
                            pts.append(pt)
                            src = qT_sb[pr]
                            nc.tensor.matmul(
                                sps[:, off:SPAN],
                                kT2_sb[0:64, 128 * j : 128 * (j + 1)],
                                src[0:64, q0 + off : q0 + SPAN],
                                start=True,
                                stop=True,
                            )
                            nc.tensor.matmul(
                                sps[:, SPAN + off : 2 * SPAN],
                                kT2_sb[64:128, 128 * j : 128 * (j + 1)],
                                src[64:128, q0 + off : q0 + SPAN],
                                start=True,
                                stop=True,
                            )
                            # exp over the two valid column blocks (strided AP)
                            nc.scalar.activation(
                                pt[:].rearrange("p (h q) -> p h q", h=2)[
                                    :, :, off:SPAN
                                ],
                                sps[:].rearrange("p (h q) -> p h q", h=2)[
                                    :, :, off:SPAN
                                ],
                                EXP,
                            )
                            if jj >= 0:
                                # causal triangle on the diagonal 128-block
                                nc.gpsimd.affine_select(
                                    pt[:].rearrange("p (h q) -> p h q", h=2)[
                                        :, :, off : off + 128
                                    ],
                                    pt[:].rearrange("p (h q) -> p h q", h=2)[
                                        :, :, off : off + 128
                                    ],
                                    pattern=[[0, 2], [1, 128]],
                                    compare_op=mybir.AluOpType.is_ge,
                                    fill=0.0,
                                    base=0,
                                    channel_multiplier=-1,
                                )
                        if prev_pv is not None:
                            emit_pv(*prev_pv)
                        prev_pv = (j, pts, off)
                    emit_pv(*prev_pv)

                    # deferred output projection of the previous span (its
                    # AllGather has been overlapping this span's attention)
                    if pending_oproj is not None:
                        pending_oproj()
                        pending_oproj = None

                    # normalization: recip of denominators via exp(-ln(d)),
                    # broadcast over 64 partitions with a K=1 matmul
                    lnb = work.tile([1, 4 * SPAN], F32, tag="lnb", name=f"lnb{J}")
                    nc.scalar.activation(lnb[:], opsum[64:65, :], LN)
                    recip = work.tile([1, 4 * SPAN], BF, tag="recip", name=f"recip{J}")
                    nc.scalar.activation(recip[:], lnb[:], EXP, scale=-1.0)

                    agin = dramp.tile([256, SPAN], BF, name=f"agin{J}")
                    agout = dramp.tile([4 * 256, SPAN], BF, name=f"agout{J}")
                    for h in range(4):
                        bc = psS.tile([64, SPAN], F32, tag="s", name=f"bc{J}_{h}")
                        nc.tensor.matmul(
                            bc[:],
                            ones_sb[:],
                            recip[0:1, SPAN * h : SPAN * (h + 1)],
                            start=True,
                            stop=True,
                        )
                        bcs = work.tile([64, SPAN], BF, tag="bcs", name=f"bcs{J}_{h}")
                        nc.vector.tensor_copy(bcs[:], bc[:])
                        onrm = work.tile([64, SPAN], BF, tag="onrm", name=f"on{J}_{h}")
                        nc.vector.tensor_mul(
                            onrm[:], opsum[0:64, SPAN * h : SPAN * (h + 1)], bcs[:]
                        )
                        nc.sync.dma_start(agin[64 * h : 64 * (h + 1), :], onrm[:])

                    nc.gpsimd.collective_compute(
                        "AllGather",
                        mybir.AluOpType.bypass,
                        replica_groups=rg,
                        ins=[agin[:].opt()],
                        outs=[agout[:].opt()],
                    )
                    ofull = work.tile(
                        [128, NCH, SPAN], BF, tag="ofull", name=f"of{J}"
                    )
                    for k in range(NCH):
                        nc.sync.dma_start(
                            ofull[:, k, :], agout[128 * k : 128 * (k + 1), :]
                        )

                    def make_oproj(J=J, q0=q0, ofull=ofull):
                        def _emit():
                            for half in range(2):
                                po = psS.tile(
                                    [128, SPAN], F32, tag="s", name=f"po{J}_{half}"
                                )
                                for k in range(NCH):
                                    nc.tensor.matmul(
                                        po[:],
                                        wo_sb[k][:, 128 * half : 128 * (half + 1)],
                                        ofull[:, k, :],
                                        start=(k == 0),
                                        stop=(k == NCH - 1),
                                    )
                                outT = work.tile(
                                    [128, SPAN], F32, tag="outT", name=f"ot{J}_{half}"
                                )
                                nc.vector.tensor_copy(outT[:], po[:])
                                nc.sync.dma_start(
                                    out[128 * half : 128 * (half + 1), q0 : q0 + SPAN],
                                    outT[:],
                                )

                        return _emit

                    pending_oproj = make_oproj()

                pending_oproj()

    nc.finalize()
    return nc


def _host_inputs(x, cos, sin, wq, wk, wv, wo):
    cosT = np.ascontiguousarray(cos.T).astype(np.float32)  # [64, S]
    sinT = np.ascontiguousarray(sin.T).astype(np.float32)
    s1n = np.concatenate([-sinT[0:32], sinT[32:64]], axis=0)  # [64, S]
    c2n = np.concatenate([cosT, cosT], axis=0)  # [128, S]
    s2n = np.concatenate([s1n, s1n], axis=0)
    ident = np.eye(128, dtype=BF16)

    in_maps = []
    for c in range(N_CORES):
        b, g = c // 4, c % 4
        xT = np.ascontiguousarray(x[b].T).astype(BF16)
        wq_c = np.ascontiguousarray(wq[:, 256 * g : 256 * (g + 1)] / 8.0).astype(BF16)
        wkv_c = np.ascontiguousarray(
            np.concatenate(
                [wk[:, 64 * g : 64 * (g + 1)], wv[:, 64 * g : 64 * (g + 1)]], axis=1
            )
        ).astype(BF16)
        wo_c = np.ascontiguousarray(wo[:, 256 * g : 256 * (g + 1)]).astype(BF16)
        in_maps.append(
            {
                "xT": xT,
                "wq": wq_c,
                "wkv": wkv_c,
                "wo": wo_c,
                "c2": c2n,
                "s2": s2n,
                "c1": cosT,
                "s1": s1n,
                "ident": ident,
            }
        )
    return in_maps


def kernel(x, cos, sin, wq, wk, wv, wo):
    if "nc" not in _CACHED:
        _CACHED["nc"] = _build_nc()
    nc = _CACHED["nc"]
    in_maps = _host_inputs(
        np.asarray(x, np.float32),
        np.asarray(cos, np.float32),
        np.asarray(sin, np.float32),
        np.asarray(wq, np.float32),
        np.asarray(wk, np.float32),
        np.asarray(wv, np.float32),
        np.asarray(wo, np.float32),
    )
    res = run_bass_kernel_spmd(
        nc, in_maps, core_ids=list(range(N_CORES)), trace=TRACE
    )
    _CACHED["last_result"] = res
    out = np.empty((B, S, HID), dtype=np.float32)
    for c in range(N_CORES):
        b, g = c // 4, c % 4
        out[b, :, 256 * g : 256 * (g + 1)] = res.results[c]["out"].T
    return out


# revision 8
# speedup vs baseline: 1.5418x; 1.1620x over previous
"""GQA attention block (RoPE + causal attention + output proj) on 8 TRN2 NeuronCores.

Sharding: batch (B=2) x kv-head-group (KVH=4) -> 8 cores.
Core c handles batch b=c//4, kv group g=c%4 (q heads 4g..4g+3, kv head g).
Per-core tensor-parallel attention; AllGather of per-head outputs within each
batch's 4-core group; column-split wo after the gather.

All matmuls run in bf16 (fp32 PSUM accumulation). Layouts are transposed
([feature, token]) so Q/K/V projections, scores (computed as S^T = K-stationary),
and PV all feed the TensorEngine without transposes; softmax runs without
max-subtraction (logits are provably small for this problem's scale).

Pipelining: PV runs one k-tile behind scores/exp; each span's output projection
is deferred until after the next span's attention so the AllGather latency hides
behind compute.
"""

import sys

if "/opt/trn_rl_repo" not in sys.path:
    sys.path.insert(0, "/opt/trn_rl_repo")

import numpy as np
import ml_dtypes

import concourse.bass as bass
import concourse.mybir as mybir
import concourse.tile as tile
from concourse import bacc
from concourse.bass_utils import run_bass_kernel_spmd

BF16 = ml_dtypes.bfloat16

B, S, HID = 2, 2048, 1024
H, KVH, D = 16, 4, 64
G = H // KVH
N_CORES = 8
SPAN = 512
NSPAN = S // SPAN  # 4
NCH = HID // 128  # 8 contraction chunks
NKT = S // 128  # 16 k-tiles
F32 = mybir.dt.float32
BF = mybir.dt.bfloat16

TRACE = False
_CACHED = {}


def _build_nc():
    nc = bacc.Bacc("TRN2", target_bir_lowering=False, debug=False, num_devices=N_CORES)

    xT = nc.dram_tensor("xT", [HID, S], BF, kind="ExternalInput")
    wq = nc.dram_tensor("wq", [HID, 256], BF, kind="ExternalInput")
    wkv = nc.dram_tensor("wkv", [HID, 128], BF, kind="ExternalInput")
    wo = nc.dram_tensor("wo", [HID, 256], BF, kind="ExternalInput")
    c2 = nc.dram_tensor("c2", [128, S], BF, kind="ExternalInput")
    s2 = nc.dram_tensor("s2", [128, S], BF, kind="ExternalInput")
    c1 = nc.dram_tensor("c1", [64, S], BF, kind="ExternalInput")
    s1 = nc.dram_tensor("s1", [64, S], BF, kind="ExternalInput")
    ident = nc.dram_tensor("ident", [128, 128], BF, kind="ExternalInput")
    out = nc.dram_tensor("out", [256, S], F32, kind="ExternalOutput")

    EXP = mybir.ActivationFunctionType.Exp
    LN = mybir.ActivationFunctionType.Ln

    with tile.TileContext(nc) as tc:
        with (
            tc.tile_pool(name="main", bufs=1) as main,
            tc.tile_pool(name="dramp", bufs=1, space="DRAM") as dramp,
        ):
            # ---- persistent SBUF; per-chunk input tiles so compute can start
            # as soon as each chunk's DMA lands ----
            xT_sb = [main.tile([128, S], BF, name=f"xT{k}") for k in range(NCH)]
            wq_sb = [main.tile([128, 256], BF, name=f"wq{k}") for k in range(NCH)]
            wkv_sb = [main.tile([128, 128], BF, name=f"wkv{k}") for k in range(NCH)]
            wo_sb = [main.tile([128, 256], BF, name=f"wo{k}") for k in range(NCH)]
            c2_sb = main.tile([128, S], BF, name="c2_sb")
            s2_sb = main.tile([128, S], BF, name="s2_sb")
            c1_sb = main.tile([64, S], BF, name="c1_sb")
            s1_sb = main.tile([64, S], BF, name="s1_sb")
            ident_sb = main.tile([128, 128], BF, name="ident_sb")
            qT0_sb = main.tile([128, S], BF, name="qT0_sb")
            qT1_sb = main.tile([128, S], BF, name="qT1_sb")
            kT2_sb = main.tile([128, S], BF, name="kT2_sb")
            vT_sb = main.tile([64, S], BF, name="vT_sb")
            vaug_sb = main.tile([128, NKT, 65], BF, name="vaug_sb")
            ones_sb = main.tile([1, 64], BF, name="ones_sb")
            qT_sb = [qT0_sb, qT1_sb]

            # tiny warmup AllGather: absorbs ncfw init + inter-core alignment
            wuin = dramp.tile([128, 16], BF, name="wuin")
            wuout = dramp.tile([512, 16], BF, name="wuout")
            nc.gpsimd.collective_compute(
                "AllGather",
                mybir.AluOpType.bypass,
                replica_groups=[[0, 1, 2, 3], [4, 5, 6, 7]],
                ins=[wuin[:].opt()],
                outs=[wuout[:].opt()],
            )
            for k in range(NCH):
                nc.sync.dma_start(wkv_sb[k][:], wkv[128 * k : 128 * k + 128, :])
            for k in range(NCH):
                nc.sync.dma_start(xT_sb[k][:], xT[128 * k : 128 * k + 128, :])
            for k in range(NCH):
                nc.sync.dma_start(wq_sb[k][:], wq[128 * k : 128 * k + 128, :])
            nc.sync.dma_start(c1_sb[:], c1[:])
            nc.sync.dma_start(s1_sb[:], s1[:])
            nc.sync.dma_start(c2_sb[:], c2[:])
            nc.sync.dma_start(s2_sb[:], s2[:])
            nc.sync.dma_start(ident_sb[:], ident[:])
            for k in range(NCH):
                nc.sync.dma_start(wo_sb[k][:], wo[128 * k : 128 * k + 128, :])
            nc.vector.memset(ones_sb[:], 1.0)

            # ---- phase 1: projections (transposed layout) + RoPE; KV first so
            # the V-transpose can run while the Q projections are still going ----
            HS = S // 2  # phase-1 half-sequence granularity (2 PSUM banks)
            with (
                tc.tile_pool(name="psA", bufs=2, space="PSUM") as psA,
                tc.tile_pool(name="ropep", bufs=2) as ropep,
                tc.tile_pool(name="psT", bufs=2, space="PSUM") as psT,
            ):
                for hf in range(2):
                    f0 = HS * hf
                    kvp = psA.tile([128, HS], F32, tag="qkv", name=f"kvp{hf}")
                    for sp in range(2):
                        for k in range(NCH):
                            nc.tensor.matmul(
                                kvp[:, SPAN * sp : SPAN * (sp + 1)],
                                wkv_sb[k][:],
                                xT_sb[k][:, f0 + SPAN * sp : f0 + SPAN * (sp + 1)],
                                start=(k == 0),
                                stop=(k == NCH - 1),
                            )
                    kb = ropep.tile([64, HS], BF, tag="kb", name=f"kb{hf}")
                    nc.scalar.copy(kb[:], kvp[0:64, :])
                    nc.scalar.copy(vT_sb[:, f0 : f0 + HS], kvp[64:128, :])
                    tcosk = ropep.tile([64, HS], BF, tag="tcos", name=f"tcosk{hf}")
                    tsink = ropep.tile([64, HS], BF, tag="tsin", name=f"tsink{hf}")
                    nc.vector.tensor_mul(tcosk[:], kb[:], c1_sb[:, f0 : f0 + HS])
                    for dst, src in ((0, 32), (32, 0)):
                        nc.vector.tensor_mul(
                            tsink[dst : dst + 32, :],
                            kb[src : src + 32, :],
                            s1_sb[src : src + 32, f0 : f0 + HS],
                        )
                    nc.vector.tensor_add(
                        kT2_sb[0:64, f0 : f0 + HS], tcosk[:], tsink[:]
                    )
                    nc.vector.tensor_copy(
                        kT2_sb[64:128, f0 : f0 + HS], kT2_sb[0:64, f0 : f0 + HS]
                    )
                    # V transpose to [token, d] for this half
                    for t in range(8 * hf, 8 * hf + 8):
                        trp = psT.tile([128, 64], BF, tag="tr", name=f"tr{t}")
                        nc.tensor.transpose(
                            trp[:],
                            vT_sb[:, 128 * t : 128 * (t + 1)],
                            ident_sb[0:64, 0:64],
                        )
                        nc.vector.tensor_copy(vaug_sb[:, t, 0:64], trp[:])
                nc.vector.memset(vaug_sb[:, :, 64:65], 1.0)

                for p in range(2):
                    for hf in range(2):
                        f0 = HS * hf
                        qp = psA.tile([128, HS], F32, tag="qkv", name=f"qp{p}_{hf}")
                        for sp in range(2):
                            for k in range(NCH):
                                nc.tensor.matmul(
                                    qp[:, SPAN * sp : SPAN * (sp + 1)],
                                    wq_sb[k][:, 128 * p : 128 * (p + 1)],
                                    xT_sb[k][:, f0 + SPAN * sp : f0 + SPAN * (sp + 1)],
                                    start=(k == 0),
                                    stop=(k == NCH - 1),
                                )
                        qb = ropep.tile([128, HS], BF, tag="qb", name=f"qb{p}{hf}")
                        nc.scalar.copy(qb[:], qp[:])
                        tcos = ropep.tile([128, HS], BF, tag="tcos", name=f"tc{p}{hf}")
                        tsin = ropep.tile([128, HS], BF, tag="tsin", name=f"ts{p}{hf}")
                        nc.vector.tensor_mul(tcos[:], qb[:], c2_sb[:, f0 : f0 + HS])
                        for dst, src in ((0, 32), (32, 0), (64, 96), (96, 64)):
                            nc.vector.tensor_mul(
                                tsin[dst : dst + 32, :],
                                qb[src : src + 32, :],
                                s2_sb[src : src + 32, f0 : f0 + HS],
                            )
                        nc.vector.tensor_add(
                            qT_sb[p][:, f0 : f0 + HS], tcos[:], tsin[:]
                        )

            # ---- phase 3: attention spans, AllGather, output projection ----
            with (
                tc.tile_pool(name="psS", bufs=2, space="PSUM") as psS,
                tc.tile_pool(name="psO", bufs=1, space="PSUM") as psO,
                tc.tile_pool(name="pp", bufs=5) as pp,
                tc.tile_pool(name="work", bufs=2) as work,
            ):
                rg = [[0, 1, 2, 3], [4, 5, 6, 7]]
                pending_oproj = []

                for J in range(NSPAN):
                    q0 = SPAN * J
                    nkt_j = 4 * (J + 1)
                    opsum = psO.tile([128, 4 * SPAN], F32, tag="o", name=f"opsum{J}")

                    prev_pv = None  # (j, [pt_pr0, pt_pr1], col offset)

                    def emit_pv(j, pts, off):
                        for pr in range(2):
                            for hh in range(2):
                                h = 2 * pr + hh
                                nc.tensor.matmul(
                                    opsum[0:65, SPAN * h + off : SPAN * (h + 1)],
                                    vaug_sb[:, j, :],
                                    pts[pr][:, SPAN * hh + off : SPAN * (hh + 1)],
                                    start=(j == 0),
                                    stop=(j == nkt_j - 1),
                                )

                    for j in range(nkt_j):
                        jj = j - 4 * J  # >= 0 on causal-boundary k-tiles
                        off = 128 * jj if jj > 0 else 0
                        pts = []
                        for pr in range(2):
                            sps = psS.tile(
                                [128, 2 * SPAN], F32, tag="s", name=f"s{J}_{j}_{pr}"
                            )
                            pt = pp.tile(
                                [128, 2 * SPAN], BF, tag="p", name=f"p{J}_{j}_{pr}"
                            )
                            pts.append(pt)
                            src = qT_sb[pr]
                            nc.tensor.matmul(
                                sps[:, off:SPAN],
                                kT2_sb[0:64, 128 * j : 128 * (j + 1)],
                                src[0:64, q0 + off : q0 + SPAN],
                                start=True,
                                stop=True,
                            )
                            nc.tensor.matmul(
                                sps[:, SPAN + off : 2 * SPAN],
                                kT2_sb[64:128, 128 * j : 128 * (j + 1)],
                                src[64:128, q0 + off : q0 + SPAN],
                                start=True,
                                stop=True,
                            )
                            # exp over the two valid column blocks (strided AP)
                            nc.scalar.activation(
                                pt[:].rearrange("p (h q) -> p h q", h=2)[
                                    :, :, off:SPAN
                                ],
                                sps[:].rearrange("p (h q) -> p h q", h=2)[
                                    :, :, off:SPAN
                                ],
                                EXP,
                            )
                            if jj >= 0:
                                # causal triangle on the diagonal 128-block
                                nc.gpsimd.affine_select(
                                    pt[:].rearrange("p (h q) -> p h q", h=2)[
                                        :, :, off : off + 128
                                    ],
                                    pt[:].rearrange("p (h q) -> p h q", h=2)[
                                        :, :, off : off + 128
                                    ],
                                    pattern=[[0, 2], [1, 128]],
                                    compare_op=mybir.AluOpType.is_ge,
                                    fill=0.0,
                                    base=0,
                                    channel_multiplier=-1,
                                )
                        if prev_pv is not None:
                            emit_pv(*prev_pv)
                        prev_pv = (j, pts, off)
                    emit_pv(*prev_pv)

                    # normalization: recip of denominators via exp(-ln(d)),
                    # broadcast over 64 partitions with a K=1 matmul
                    lnb = work.tile([1, 4 * SPAN], F32, tag="lnb", name=f"lnb{J}")
                    nc.scalar.activation(lnb[:], opsum[64:65, :], LN)
                    recip = work.tile([1, 4 * SPAN], BF, tag="recip", name=f"recip{J}")
                    nc.scalar.activation(recip[:], lnb[:], EXP, scale=-1.0)

                    # last span: two half-width AllGathers to shorten the tail
                    chunks = (
                        [(0, SPAN)] if J < NSPAN - 1 else [(0, SPAN // 2), (SPAN // 2, SPAN // 2)]
                    )
                    new_oproj = []
                    for c0, cw in chunks:
                        agin = dramp.tile([256, cw], BF, name=f"agin{J}_{c0}")
                        agout = dramp.tile([4 * 256, cw], BF, name=f"agout{J}_{c0}")
                        for h in range(4):
                            bc = psS.tile([64, cw], F32, tag="s", name=f"bc{J}_{h}_{c0}")
                            nc.tensor.matmul(
                                bc[:],
                                ones_sb[:],
                                recip[0:1, SPAN * h + c0 : SPAN * h + c0 + cw],
                                start=True,
                                stop=True,
                            )
                            bcs = work.tile([64, cw], BF, tag="bcs", name=f"bcs{J}_{h}_{c0}")
                            nc.vector.tensor_copy(bcs[:], bc[:])
                            onrm = work.tile([64, cw], BF, tag="onrm", name=f"on{J}_{h}_{c0}")
                            nc.vector.tensor_mul(
                                onrm[:],
                                opsum[0:64, SPAN * h + c0 : SPAN * h + c0 + cw],
                                bcs[:],
                            )
                            nc.sync.dma_start(agin[64 * h : 64 * (h + 1), :], onrm[:])

                        nc.gpsimd.collective_compute(
                            "AllGather",
                            mybir.AluOpType.bypass,
                            replica_groups=rg,
                            ins=[agin[:].opt()],
                            outs=[agout[:].opt()],
                        )
                        ofull = work.tile(
                            [128, NCH, cw], BF, tag="ofull", bufs=3, name=f"of{J}_{c0}"
                        )
                        for k in range(NCH):
                            nc.sync.dma_start(
                                ofull[:, k, :], agout[128 * k : 128 * (k + 1), :]
                            )

                        def make_oproj(J=J, q0=q0, c0=c0, cw=cw, ofull=ofull):
                            def _emit():
                                for half in range(2):
                                    po = psS.tile(
                                        [128, cw], F32, tag="s", name=f"po{J}_{half}_{c0}"
                                    )
                                    for k in range(NCH):
                                        nc.tensor.matmul(
                                            po[:],
                                            wo_sb[k][:, 128 * half : 128 * (half + 1)],
                                            ofull[:, k, :],
                                            start=(k == 0),
                                            stop=(k == NCH - 1),
                                        )
                                    outT = work.tile(
                                        [128, cw], F32, tag="outT", name=f"ot{J}_{half}_{c0}"
                                    )
                                    nc.vector.tensor_copy(outT[:], po[:])
                                    nc.sync.dma_start(
                                        out[
                                            128 * half : 128 * (half + 1),
                                            q0 + c0 : q0 + c0 + cw,
                                        ],
                                        outT[:],
                                    )

                            return _emit

                        new_oproj.append(make_oproj())

                    # deferred output projection of the previous span (its
                    # AllGather has been overlapping this span's attention)
                    for fn in pending_oproj:
                        fn()
                    pending_oproj = new_oproj

                for fn in pending_oproj:
                    fn()

    nc.finalize()
    return nc


def _host_inputs(x, cos, sin, wq, wk, wv, wo):
    cosT = np.ascontiguousarray(cos.T).astype(np.float32)  # [64, S]
    sinT = np.ascontiguousarray(sin.T).astype(np.float32)
    s1n = np.concatenate([-sinT[0:32], sinT[32:64]], axis=0)  # [64, S]
    c2n = np.concatenate([cosT, cosT], axis=0).astype(BF16)  # [128, S]
    # partition-swapped: row p holds the sin factor for the partner row p^32,
    # so both DVE operands read from the same base partition
    s1w = np.concatenate([s1n[32:64], s1n[0:32]], axis=0)
    s2w = np.concatenate([s1w, s1w], axis=0).astype(BF16)
    cosT = cosT.astype(BF16)
    s1w = s1w.astype(BF16)
    ident = np.eye(128, dtype=BF16)

    in_maps = []
    for c in range(N_CORES):
        b, g = c // 4, c % 4
        xT = np.ascontiguousarray(x[b].T).astype(BF16)
        wq_c = np.ascontiguousarray(wq[:, 256 * g : 256 * (g + 1)] / 8.0).astype(BF16)
        wkv_c = np.ascontiguousarray(
            np.concatenate(
                [wk[:, 64 * g : 64 * (g + 1)], wv[:, 64 * g : 64 * (g + 1)]], axis=1
            )
        ).astype(BF16)
        wo_c = np.ascontiguousarray(wo[:, 256 * g : 256 * (g + 1)]).astype(BF16)
        in_maps.append(
            {
                "xT": xT,
                "wq": wq_c,
                "wkv": wkv_c,
                "wo": wo_c,
                "c2": c2n,
                "s2": s2w,
                "c1": cosT,
                "s1": s1w,
                "ident": ident,
            }
        )
    return in_maps


def kernel(x, cos, sin, wq, wk, wv, wo):
    if "nc" not in _CACHED:
        _CACHED["nc"] = _build_nc()
    nc = _CACHED["nc"]
    in_maps = _host_inputs(
        np.asarray(x, np.float32),
        np.asarray(cos, np.float32),
        np.asarray(sin, np.float32),
        np.asarray(wq, np.float32),
        np.asarray(wk, np.float32),
        np.asarray(wv, np.float32),
        np.asarray(wo, np.float32),
    )
    res = run_bass_kernel_spmd(
        nc, in_maps, core_ids=list(range(N_CORES)), trace=TRACE
    )
    _CACHED["last_result"] = res
    out = np.empty((B, S, HID), dtype=np.float32)
    for c in range(N_CORES):
        b, g = c // 4, c % 4
        out[b, :, 256 * g : 256 * (g + 1)] = res.results[c]["out"].T
    return out
